# revision 6
# baseline (speedup 1.0000x reference)
"""MinLSTM Bass/Trainium2 kernel.

Math (exact reformulation of the log-space reference into linear space):
  proj = x @ W_proj.T -> [h~ | ig | fg] each [S, H]
  a = 1+exp(-fg), b = 1+exp(-ig)
  f = b/(a+b) = sigmoid(softplus(-ig) - softplus(-fg))   (forget gate)
  g = relu(h~) + min(sigmoid(h~), 0.5)                   (= log-space g, linearized)
  h_t = f_t * h_{t-1} + (1-f_t) * g_t                    (tensor_tensor_scan)
  out = h @ W_out.T ; new_hidden = h[-1]

Sharding: data-parallel over batch, one batch element per NeuronCore (8 cores).
"""

import numpy as np

B, S, D = 8, 4096, 512
H = 1024
G3 = 3 * H
P = 128
SC = 512                 # seq chunk (free dim of matmul / gate tiles)
NK = S // SC             # 8 seq chunks
JH = H // P              # 8 hidden chunks
DC = D // P              # 4 contraction chunks for proj
N_CORES = 8

_cache = {}


def _build_nc(mm_f32r=True):
    import concourse.bacc as bacc
    import concourse.bass as bass
    import concourse.tile as tile
    from concourse import mybir
    from concourse.masks import make_identity

    f32 = mybir.dt.float32
    f32r = mybir.dt.float32r
    AF = mybir.ActivationFunctionType
    OP = mybir.AluOpType
    mmdt = f32r if mm_f32r else f32

    nc = bacc.Bacc("TRN2", target_bir_lowering=False)

    x = nc.dram_tensor("x", [S, D], f32, kind="ExternalInput").ap()
    wpT = nc.dram_tensor("wpT", [D, G3], mmdt, kind="ExternalInput").ap()
    woT = nc.dram_tensor("woT", [H, D], mmdt, kind="ExternalInput").ap()
    h0T = nc.dram_tensor("h0T", [P, JH], f32, kind="ExternalInput").ap()
    out = nc.dram_tensor("out", [S, D], f32, kind="ExternalOutput").ap()
    hNT = nc.dram_tensor("hNT", [P, JH], mmdt, kind="ExternalOutput").ap()

    with tile.TileContext(nc) as tc:
        from contextlib import ExitStack

        with ExitStack() as ctx:
            consts = ctx.enter_context(tc.tile_pool(name="consts", bufs=1))
            weights = ctx.enter_context(tc.tile_pool(name="weights", bufs=1))
            xtp = ctx.enter_context(tc.tile_pool(name="xT", bufs=1))
            gate = ctx.enter_context(tc.tile_pool(name="gate", bufs=2))
            hpool = ctx.enter_context(tc.tile_pool(name="h", bufs=2))
            obuf = ctx.enter_context(tc.tile_pool(name="obuf", bufs=2))

            ident = consts.tile([P, P], f32)
            make_identity(nc, ident)

            # Weights: wpT [D, 3H] -> SBUF [128, DC, 3H]; woT [H, D] -> [128, JH, D]
            wp_sb = weights.tile([P, DC, G3], mmdt)
            nc.sync.dma_start(wp_sb, wpT.rearrange("(dc p) h -> p dc h", p=P))
            wo_sb = weights.tile([P, JH, D], mmdt)
            nc.sync.dma_start(wo_sb, woT.rearrange("(j p) d -> p j d", p=P))
            h0_sb = consts.tile([P, JH], f32)
            nc.sync.dma_start(h0_sb, h0T)

            # Pre-phase: transpose x [S, D] -> xT_sb [128, DC, S]
            xT_sb = xtp.tile([P, DC, S], mmdt)
            with tc.tile_pool(name="xstage", bufs=3) as xsp, tc.tile_pool(
                name="psum_pre", bufs=2, space="PSUM"
            ) as ppre:
                pt = [None] * DC
                for st in range(S // P):
                    xs = xsp.tile([P, D], f32, tag="xs")
                    nc.sync.dma_start(xs, x[st * P : (st + 1) * P, :])
                    q = st % 4
                    for di in range(DC):
                        if q == 0:
                            pt[di] = ppre.tile(
                                [P, SC], f32, tag=f"tp{di}", name=f"tp{di}"
                            )
                        nc.tensor.transpose(
                            pt[di][:, q * P : (q + 1) * P],
                            xs[:, di * P : (di + 1) * P],
                            ident,
                        )
                    if q == 3:
                        sc4 = st // 4
                        for di in range(DC):
                            nc.scalar.copy(
                                xT_sb[:, di, sc4 * SC : (sc4 + 1) * SC], pt[di]
                            )

            psum_main = ctx.enter_context(
                tc.tile_pool(name="psum_main", bufs=2, space="PSUM")
            )
            psum_out = ctx.enter_context(
                tc.tile_pool(name="psum_out", bufs=2, space="PSUM")
            )

            h_prev = [None] * JH
            for k in range(NK):
                s0 = k * SC
                for j in range(JH):
                    ph_t = psum_main.tile([P, SC], f32, tag="ph")
                    pi_t = psum_main.tile([P, SC], f32, tag="pi")
                    pf_t = psum_main.tile([P, SC], f32, tag="pf")
                    for pt, jc in ((ph_t, j), (pi_t, JH + j), (pf_t, 2 * JH + j)):
                        for di in range(DC):
                            nc.tensor.matmul(
                                pt,
                                lhsT=wp_sb[:, di, jc * P : (jc + 1) * P],
                                rhs=xT_sb[:, di, s0 : s0 + SC],
                                start=(di == 0),
                                stop=(di == DC - 1),
                            )
                    # f = sigmoid(fg) / (sigmoid(fg) + sigmoid(ig))
                    # (uses 1 + e^{-x} = 1/sigmoid(x); only Sigmoid LUT needed)
                    sgf = gate.tile([P, SC], f32, tag="sgf")
                    nc.scalar.activation(sgf, pf_t, AF.Sigmoid)
                    sgi = gate.tile([P, SC], f32, tag="sgi")
                    nc.scalar.activation(sgi, pi_t, AF.Sigmoid)
                    s_t = gate.tile([P, SC], f32, tag="s")
                    nc.vector.tensor_add(s_t, sgf, sgi)
                    r_t = gate.tile([P, SC], f32, tag="r")
                    nc.vector.reciprocal_approx_fast(r_t, s_t)
                    f_t = gate.tile([P, SC], f32, tag="f")
                    nc.vector.tensor_mul(f_t, sgf, r_t)
                    sg = gate.tile([P, SC], f32, tag="sg")
                    nc.scalar.activation(sg, ph_t, AF.Sigmoid)
                    sgm = gate.tile([P, SC], f32, tag="sgm")
                    nc.vector.tensor_scalar_min(sgm, sg, 0.5)
                    g_t = gate.tile([P, SC], f32, tag="g")
                    nc.vector.scalar_tensor_tensor(
                        g_t, in0=ph_t, scalar=0.0, in1=sgm, op0=OP.max, op1=OP.add
                    )
                    d_t = gate.tile([P, SC], f32, tag="d")
                    nc.vector.scalar_tensor_tensor(
                        d_t, in0=f_t, scalar=1.0, in1=g_t,
                        op0=OP.subtract, op1=OP.mult,
                    )
                    h_t = hpool.tile([P, SC], mmdt, tag=f"h{j}")
                    init = h0_sb[:, j : j + 1] if k == 0 else h_prev[j][:, SC - 1 : SC]
                    nc.vector.tensor_tensor_scan(
                        h_t, data0=f_t, data1=d_t, initial=init,
                        op0=OP.mult, op1=OP.subtract,
                    )
                    h_prev[j] = h_t
                for ss in range(4):
                    po = psum_out.tile([P, D], f32, tag="po")
                    for j in range(JH):
                        nc.tensor.matmul(
                            po,
                            lhsT=h_prev[j][:, ss * P : (ss + 1) * P],
                            rhs=wo_sb[:, j, :],
                            start=(j == 0),
                            stop=(j == JH - 1),
                        )
                    ob = obuf.tile([P, D], f32, tag="ob")
                    nc.scalar.copy(ob, po)
                    st = s0 + ss * P
                    nc.sync.dma_start(out[st : st + P, :], ob)
            for j in range(JH):
                nc.sync.dma_start(hNT[:, j : j + 1], h_prev[j][:, SC - 1 : SC])

    nc.compile()
    return nc


def _g_np(v):
    v = v.astype(np.float32)
    return np.where(v >= 0, v + np.float32(0.5),
                    (1.0 / (1.0 + np.exp(-v))).astype(np.float32)).astype(np.float32)


def kernel(inputs, prev_hidden, W_proj, W_out):
    from concourse.bass_utils import run_bass_kernel_spmd

    if "nc" not in _cache:
        _cache["nc"] = _build_nc()
    nc = _cache["nc"]

    x = np.ascontiguousarray(np.asarray(inputs, np.float32))        # [B, S, D]
    wpT = np.ascontiguousarray(np.asarray(W_proj, np.float32).T)    # [D, 3H]
    woT = np.ascontiguousarray(np.asarray(W_out, np.float32).T)     # [H, D]
    ph = np.asarray(prev_hidden, np.float32)                        # [B, 1, H]

    in_maps = []
    for b in range(N_CORES):
        h0T = np.ascontiguousarray(_g_np(ph[b, 0]).reshape(JH, P).T)  # [P, JH]
        in_maps.append({"x": x[b], "wpT": wpT, "woT": woT, "h0T": h0T})

    res = run_bass_kernel_spmd(nc, in_maps, core_ids=list(range(N_CORES)))
    outs = np.stack([res.results[b]["out"] for b in range(N_CORES)])  # [B, S, D]
    hN = np.stack(
        [res.results[b]["hNT"].T.reshape(1, H) for b in range(N_CORES)]
    )  # [B, 1, H]
    return outs, hN
